# revision 17
# baseline (speedup 1.0000x reference)
"""Trainium2 Bass kernel for unfolded D-ADMM logistic-regression diffusion
(gnn_message_passing, P=100 nodes on a 4-regular graph, B=128, N=784, K=10).

Strategy (fully self-contained; shapes hardcoded):
  - Shard the batch dimension B across the 8 NeuronCores (16 samples each).
    Samples never interact => zero inter-core communication.
  - On each core, put the NODE dimension on SBUF partitions, permuted so that
    color-0 nodes occupy partitions [0:50) and color-1 nodes [50:100).
  - All node-linear terms (neighbor sums + diagonal scalings, with the
    per-(iteration,node) |h| coefficients folded in) become 100x100 banded
    matrices applied by the TensorEngine per batch-sample column block.
  - The VectorEngine computes the per-(node,sample) inner products
    xa = sum_n x*a (fused multiply+accumulate) and the final combine
    a' = x * seff + psum   (one fused scalar_tensor_tensor per sample).
  - ScalarE (ACT) + GPSIMD handle the dual update mu += W3 @ a.
  - State (a, mu, x) stays resident in SBUF for all 10 iterations.
"""

import numpy as np

import concourse.bass as bass
from concourse import bacc
import concourse.mybir as mybir
import concourse.tile as tile
from concourse.bass_utils import run_bass_kernel_spmd
from concourse.tile_rust import add_dep_helper

P, B, N, K, DEG = 100, 128, 784, 10, 4
NP = 128          # padded node/partition space: evens at rows 0:50, odds at 64:114
NCORES = 8
BS = B // NCORES  # 16 samples per core
NC0 = 512         # psum chunk sizes (bank-aligned: 512 fp32 = one 2KB bank)
NC1 = N - NC0     # 272
NCOEF = 7

FP32 = mybir.dt.float32

_BUILD_CACHE = {}


def _build_bass():
    """Build the (compile-once) Bass program shared by all 8 cores."""
    if "nc" in _BUILD_CACHE:
        return _BUILD_CACHE["nc"]

    nc = bacc.Bacc("TRN2", target_bir_lowering=False, debug=False,
                   num_devices=NCORES)

    x_d = nc.dram_tensor("x_in", [NP, BS, N], FP32, kind="ExternalInput").ap()
    a_d = nc.dram_tensor("a_in", [NP, BS, N], FP32, kind="ExternalInput").ap()
    y_d = nc.dram_tensor("y_in", [NP, BS], FP32, kind="ExternalInput").ap()
    w_d = nc.dram_tensor("w_in", [NP, BS], FP32, kind="ExternalInput").ap()
    # weight mats, already transposed host-side to [j(part), K, 5, i]
    wa_d = nc.dram_tensor("wa_in", [NP, K, 5, NP], FP32, kind="ExternalInput").ap()
    band_d = nc.dram_tensor("band_in", [NP, 3, NP], FP32, kind="ExternalInput").ap()
    coef_d = nc.dram_tensor("coef_in", [NP, K, NCOEF], FP32, kind="ExternalInput").ap()
    aout_d = nc.dram_tensor("a_out", [NP, BS, N], FP32, kind="ExternalOutput").ap()
    wout_d = nc.dram_tensor("w_out", [NP, BS], FP32, kind="ExternalOutput").ap()

    add = mybir.AluOpType.add
    sub = mybir.AluOpType.subtract
    mult = mybir.AluOpType.mult

    with tile.TileContext(nc) as tc:
        with (
            tc.tile_pool(name="state", bufs=1) as state,
            tc.tile_pool(name="small", bufs=1) as small,
            tc.tile_pool(name="xascr", bufs=1) as xascr,
            tc.tile_pool(name="ps", bufs=3, space="PSUM") as psp,
            tc.tile_pool(name="omps", bufs=2, space="PSUM") as ompsp,
        ):
            X = state.tile([NP, BS, N], FP32)
            A = state.tile([NP, BS, N], FP32)
            MU = state.tile([NP, BS, N], FP32)
            WA = state.tile([NP, K, 5, NP], FP32)
            BAND = state.tile([NP, 3, NP], FP32)
            COEF = state.tile([NP, K, NCOEF], FP32)
            Y = small.tile([NP, BS], FP32)
            WOM = small.tile([NP, BS], FP32)
            LAM = small.tile([NP, BS], FP32)
            XA = small.tile([NP, BS], FP32)
            S = small.tile([NP, BS], FP32)
            SEFF = small.tile([NP, BS], FP32)
            OMV = small.tile([NP, BS], FP32)
            JSC = small.tile([NP, 80], FP32)

            # ---- loads ----
            nc.sync.dma_start(out=X[:], in_=x_d[:])
            nc.sync.dma_start(out=A[:], in_=a_d[:])
            nc.sync.dma_start(out=WA[:], in_=wa_d[:])
            nc.sync.dma_start(out=BAND[:], in_=band_d[:])
            nc.sync.dma_start(out=COEF[:], in_=coef_d[:])
            nc.sync.dma_start(out=Y[:], in_=y_d[:])
            nc.sync.dma_start(out=WOM[:], in_=w_d[:])
            nc.vector.memset(MU[:], 0.0)
            nc.vector.memset(LAM[:], 0.0)
            nc.vector.memset(JSC[:], 0.0)

            # Several ISA structs (STT, LDWEIGHTS) have a single sync-wait
            # slot, so no real instruction may be the first observer of two
            # input-DMA queues at once. Tiny per-queue "joiner" ops make each
            # engine observe every DMA semaphore one at a time first.
            # DVE joiners (ordered ahead of the sweeps via WAW on XA/S):
            nc.vector.tensor_copy(out=XA[0:1, 0:1], in_=X[0:1, 0, 0:1])
            nc.vector.tensor_copy(out=XA[0:1, 1:2], in_=A[0:1, 0, 0:1])
            nc.vector.tensor_copy(out=S[0:1, 0:1], in_=WOM[0:1, 0:1])
            nc.vector.tensor_copy(out=S[0:1, 1:2], in_=Y[0:1, 0:1])
            nc.vector.tensor_copy(out=S[0:1, 2:3], in_=COEF[0:1, 0, 0:1])
            # PE joiners (self-reads; ordering edges added to the first
            # sweep's matmuls below):
            jps = ompsp.tile([NP, BS], FP32, tag="omps")
            pe_joiners = [
                nc.tensor.matmul(jps[0:1, 0:1], WA[0:1, 0, 0, 0:1],
                                 WA[0:1, 0, 0, 0:1], start=True, stop=True),
                nc.tensor.matmul(jps[0:1, 0:1], BAND[0:1, 0, 0:1],
                                 BAND[0:1, 0, 0:1], start=True, stop=True),
                nc.tensor.matmul(jps[0:1, 0:1], WOM[0:1, 0:1],
                                 WOM[0:1, 0:1], start=True, stop=True),
                nc.tensor.matmul(jps[0:1, 0:1], A[0:1, 0, 0:1],
                                 A[0:1, 0, 0:1], start=True, stop=True),
            ]
            first_sweep_mms = []

            blocks = ((0, 50), (64, 114))

            touch_idx = [0]

            def touch(psum_ap):
                """Absorb a cross-engine (PE) wait into a plain TT (which has
                a sync-wait slot to spare) so the STT/TT that consumes
                `psum_ap` needs at most one wait. Rotating output cells and a
                static in1 keep the toucher itself at a single (PE) wait."""
                cell = touch_idx[0] % 80
                touch_idx[0] += 1
                t = nc.vector.tensor_tensor(JSC[0:1, cell:cell + 1],
                                            psum_ap[0:1, 0:1],
                                            X[0:1, 0, 0:1], add)
                return t

            for k in range(K):
                # ---------------- color sweeps ----------------
                for c, (r0, r1) in enumerate(blocks):
                    rows = slice(r0, r1)
                    m1 = 2 * c        # band+diag weights for a
                    m2 = 2 * c + 1    # mu diagonal weights

                    # omega neighbor sum (PE, 0/1 band, tiny free dim)
                    omps = ompsp.tile([NP, BS], FP32, tag="omps")
                    mm = nc.tensor.matmul(omps[:], BAND[:, c, :], WOM[:],
                                          start=True, stop=True)
                    if k == 0 and c == 0:
                        for j in pe_joiners:
                            add_dep_helper(mm.ins, j.ins, sync=False,
                                           reason="pe joiner order")

                    # xa[p, b] = sum_n X*A  for the active color rows
                    for b in range(BS):
                        scr = xascr.tile([NP, N], FP32, tag="xascr")
                        nc.vector.scalar_tensor_tensor(
                            out=scr[rows], in0=X[rows, b], scalar=1.0,
                            in1=A[rows, b], op0=mult, op1=mult,
                            accum_out=XA[rows, b:b + 1])

                    # s = xa + omega - y ; seff = -h1 * s
                    nc.vector.scalar_tensor_tensor(
                        out=S[rows], in0=XA[rows], scalar=1.0, in1=WOM[rows],
                        op0=mult, op1=add)
                    nc.vector.tensor_tensor(S[rows], S[rows], Y[rows], sub)
                    nc.vector.tensor_scalar_mul(SEFF[rows], S[rows],
                                                COEF[rows, k, 0:1])

                    # omega update:
                    # w' = cw*w + csw*s + clam*lam + cnbw*sum_w
                    t_om = touch(omps)
                    nc.vector.tensor_scalar_mul(OMV[rows], WOM[rows],
                                                COEF[rows, k, 1:2])
                    nc.vector.scalar_tensor_tensor(
                        out=OMV[rows], in0=S[rows], scalar=COEF[rows, k, 2:3],
                        in1=OMV[rows], op0=mult, op1=add)
                    if k > 0:
                        nc.vector.scalar_tensor_tensor(
                            out=OMV[rows], in0=LAM[rows],
                            scalar=COEF[rows, k, 3:4],
                            in1=OMV[rows], op0=mult, op1=add)
                    stt_om = nc.vector.scalar_tensor_tensor(
                        out=WOM[rows], in0=omps[rows],
                        scalar=COEF[rows, k, 4:5],
                        in1=OMV[rows], op0=mult, op1=add)
                    add_dep_helper(stt_om.ins, t_om.ins, sync=False,
                                   reason="stt wait absorb")

                    # a-side: psum_b = W1 @ A_b (+ W2 @ MU_b), then
                    # a'_b[rows] = X_b * seff_b + psum_b[rows]
                    for b in range(BS):
                        ps = psp.tile([NP, N], FP32, tag="ps")
                        last = k == 0
                        mm1 = nc.tensor.matmul(ps[:, 0:NC0], WA[:, k, m1, :],
                                               A[:, b, 0:NC0],
                                               start=True, stop=last)
                        mm2 = nc.tensor.matmul(ps[:, NC0:N], WA[:, k, m1, :],
                                               A[:, b, NC0:N],
                                               start=True, stop=last)
                        if k == 0 and c == 0:
                            for j in pe_joiners:
                                add_dep_helper(mm1.ins, j.ins, sync=False,
                                               reason="pe joiner order")
                                add_dep_helper(mm2.ins, j.ins, sync=False,
                                               reason="pe joiner order")
                        if k > 0:
                            nc.tensor.matmul(ps[:, 0:NC0], WA[:, k, m2, :],
                                             MU[:, b, 0:NC0],
                                             start=False, stop=True)
                            nc.tensor.matmul(ps[:, NC0:N], WA[:, k, m2, :],
                                             MU[:, b, NC0:N],
                                             start=False, stop=True)
                        t_ps = touch(ps)
                        stt_cb = nc.vector.scalar_tensor_tensor(
                            out=A[rows, b], in0=X[rows, b],
                            scalar=SEFF[rows, b:b + 1],
                            in1=ps[rows], op0=mult, op1=add)
                        add_dep_helper(stt_cb.ins, t_ps.ins, sync=False,
                                       reason="stt wait absorb")

                # ---------------- dual updates ----------------
                if k == K - 1:
                    continue  # final mu/lam are dead state
                # lam' = lam + 4*e4*omega - e4*sum_w
                omps2 = ompsp.tile([NP, BS], FP32, tag="omps")
                nc.tensor.matmul(omps2[:], BAND[:, 2, :], WOM[:],
                                 start=True, stop=True)
                t_om2 = touch(omps2)
                stt_l1 = nc.vector.scalar_tensor_tensor(
                    out=LAM[:], in0=WOM[:], scalar=COEF[:, k, 5:6],
                    in1=LAM[:], op0=mult, op1=add)
                stt_l2 = nc.vector.scalar_tensor_tensor(
                    out=LAM[:], in0=omps2[:], scalar=COEF[:, k, 6:7],
                    in1=LAM[:], op0=mult, op1=add)
                add_dep_helper(stt_l1.ins, t_om2.ins, sync=False,
                               reason="stt wait absorb")
                add_dep_helper(stt_l2.ins, t_om2.ins, sync=False,
                               reason="stt wait absorb")
                # mu += W3 @ A  (PE banded matmul, accumulated on DVE)
                for b in range(BS):
                    ps = psp.tile([NP, N], FP32, tag="ps")
                    nc.tensor.matmul(ps[:, 0:NC0], WA[:, k, 4, :],
                                     A[:, b, 0:NC0], start=True, stop=True)
                    nc.tensor.matmul(ps[:, NC0:N], WA[:, k, 4, :],
                                     A[:, b, NC0:N], start=True, stop=True)
                    t_mu = touch(ps)
                    tt_mu = nc.vector.tensor_tensor(MU[:, b], MU[:, b],
                                                    ps[:], add)
                    add_dep_helper(tt_mu.ins, t_mu.ins, sync=False,
                                   reason="tt wait absorb")

            # ---- store results ----
            nc.sync.dma_start(out=aout_d[:], in_=A[:])
            nc.sync.dma_start(out=wout_d[:], in_=WOM[:])

    nc.compile()
    _BUILD_CACHE["nc"] = nc
    return nc


def _host_prep(inputs, labels, hyp, a0, omega0, neighbors, colors):
    """Permute nodes into the padded 128-row partition space (color-0 nodes at
    rows 0:50, color-1 at rows 64:114) and fold |h| coefficients into the
    per-iteration node-linear weight matrices."""
    colors = np.asarray(colors)
    neighbors = np.asarray(neighbors)
    perm = np.concatenate([colors[0], colors[1]]).astype(np.int64)
    missing = np.setdiff1d(np.arange(P), perm)
    perm = np.concatenate([perm, missing])          # new idx -> orig node
    ncol = colors.shape[1]
    # new idx -> partition row (color blocks at aligned bases 0 and 64)
    rowmap = np.empty(P, np.int64)
    rowmap[:ncol] = np.arange(ncol)
    rowmap[ncol:2 * ncol] = 64 + np.arange(ncol)
    if 2 * ncol < P:
        rowmap[2 * ncol:] = 114 + np.arange(P - 2 * ncol)
    # orig node -> partition row
    row_of = np.empty(P, np.int64)
    row_of[perm] = rowmap
    nb_row = row_of[neighbors[perm]]                # [P, D] neighbor rows
    habs = np.abs(np.asarray(hyp, np.float64))[:, perm, :]  # [K, P, 6]
    al, be, ga, e3, e4, de = [habs[:, :, j] for j in range(6)]

    blocks = ((0, ncol), (ncol, 2 * ncol))

    WA = np.zeros((K, 5, NP, NP), np.float64)
    for k in range(K):
        for c, (r0, r1) in enumerate(blocks):
            for i in range(r0, r1):
                ri = rowmap[i]
                ab = al[k, i] * be[k, i]
                WA[k, 2 * c, ri, ri] += 1.0 - DEG * ab
                for rj in nb_row[i]:
                    WA[k, 2 * c, rj, ri] += ab
                WA[k, 2 * c + 1, ri, ri] = -DEG * be[k, i]
        for i in range(P):
            ri = rowmap[i]
            WA[k, 4, ri, ri] += DEG * e3[k, i]
            for rj in nb_row[i]:
                WA[k, 4, rj, ri] -= e3[k, i]

    BAND = np.zeros((3, NP, NP), np.float64)
    for c, (r0, r1) in enumerate(blocks):
        for i in range(r0, r1):
            for rj in nb_row[i]:
                BAND[c, rj, rowmap[i]] += 1.0
    for i in range(P):
        for rj in nb_row[i]:
            BAND[2, rj, rowmap[i]] += 1.0

    COEF = np.zeros((NP, K, NCOEF), np.float64)
    COEF[rowmap, :, 0] = -be.T
    COEF[rowmap, :, 1] = (1.0 - DEG * ga * de).T
    COEF[rowmap, :, 2] = -de.T
    COEF[rowmap, :, 3] = (-DEG * de).T
    COEF[rowmap, :, 4] = (ga * de).T
    COEF[rowmap, :, 5] = (DEG * e4).T
    COEF[rowmap, :, 6] = -e4.T

    Xp = np.zeros((NP, B, N), np.float32)
    Xp[rowmap] = np.asarray(inputs)[perm]
    A0p = np.zeros((NP, B, N), np.float32)
    A0p[rowmap] = np.asarray(a0)[perm]
    Yp = np.zeros((NP, B), np.float32)
    Yp[rowmap] = np.asarray(labels)[perm]
    W0p = np.zeros((NP, B), np.float32)
    W0p[rowmap] = np.asarray(omega0)[perm]
    WAh = np.ascontiguousarray(WA.transpose(2, 0, 1, 3), np.float32)
    BANDh = np.ascontiguousarray(BAND.transpose(1, 0, 2), np.float32)
    COEFh = np.ascontiguousarray(COEF, np.float32)
    return perm, rowmap, WAh, BANDh, COEFh, Xp, A0p, Yp, W0p


def kernel(inputs, labels, hyp, a0, omega0, neighbors, colors):
    perm, rowmap, WAh, BANDh, COEFh, Xp, A0p, Yp, W0p = _host_prep(
        inputs, labels, hyp, a0, omega0, neighbors, colors)
    nc = _build_bass()

    in_maps = []
    for core in range(NCORES):
        bsl = slice(core * BS, (core + 1) * BS)
        in_maps.append({
            "x_in": np.ascontiguousarray(Xp[:, bsl, :]),
            "a_in": np.ascontiguousarray(A0p[:, bsl, :]),
            "y_in": np.ascontiguousarray(Yp[:, bsl]),
            "w_in": np.ascontiguousarray(W0p[:, bsl]),
            "wa_in": WAh,
            "band_in": BANDh,
            "coef_in": COEFh,
        })

    res = run_bass_kernel_spmd(nc, in_maps, core_ids=list(range(NCORES)))

    a_full = np.empty((P, B, N), np.float32)
    om_full = np.empty((P, B), np.float32)
    for core, r in enumerate(res.results):
        bsl = slice(core * BS, (core + 1) * BS)
        a_full[perm, bsl, :] = r["a_out"][rowmap]
        om_full[perm, bsl] = r["w_out"][rowmap]
    return a_full, om_full


# revision 20
# speedup vs baseline: 4760.6686x; 4760.6686x over previous
"""Trainium2 Bass kernel for unfolded D-ADMM logistic-regression diffusion
(gnn_message_passing, P=100 nodes on a 4-regular graph, B=128, N=784, K=10).

Strategy (fully self-contained; shapes hardcoded):
  - Shard the batch dimension B across the 8 NeuronCores (16 samples each).
    Samples never interact => zero inter-core communication.
  - On each core, put the NODE dimension on SBUF partitions: color-0 nodes at
    rows 0:50, color-1 nodes at rows 64:114 (32-aligned bases).
  - All node-linear terms (neighbor sums + diagonal scalings, with the
    per-(iteration,node) |h| coefficients folded in) become 128x128 banded
    matrices applied by the TensorEngine per batch-sample column block.
    State (a, mu) and the weights use float32r so the PE runs at full rate.
  - The VectorEngine computes per-(node,sample) inner products
    xa = sum_n x*a (fused multiply+accumulate via scalar_tensor_tensor) and
    the combine a' = x*seff + psum (one fused STT per sample). The odd-color
    xa product runs on GPSIMD with the reduction on ScalarE to offload DVE.
  - The dual update mu += W3 @ a is spread across DVE / ACT+GPSIMD.
  - All state stays resident in SBUF for all 10 iterations.
"""

import numpy as np

import concourse.bass as bass
from concourse import bacc
import concourse.mybir as mybir
import concourse.tile as tile
from concourse.bass_utils import run_bass_kernel_spmd

P, B, N, K, DEG = 100, 128, 784, 10, 4
NP = 128          # padded node/partition space: evens at 0:50, odds at 64:114
NCORES = 8
BS = B // NCORES  # 16 samples per core
NC0 = 512         # psum chunk sizes (bank-aligned: 512 fp32 = one 2KB bank)
NCOEF = 7

FP32 = mybir.dt.float32
F32R = mybir.dt.float32r

_BUILD_CACHE = {}


def _build_bass():
    """Build the (compile-once) Bass program shared by all 8 cores."""
    if "nc" in _BUILD_CACHE:
        return _BUILD_CACHE["nc"]

    nc = bacc.Bacc("TRN2", target_bir_lowering=False, debug=False,
                   num_devices=NCORES)

    x_d = nc.dram_tensor("x_in", [NP, BS, N], FP32, kind="ExternalInput").ap()
    a_d = nc.dram_tensor("a_in", [NP, BS, N], F32R, kind="ExternalInput").ap()
    y_d = nc.dram_tensor("y_in", [NP, BS], FP32, kind="ExternalInput").ap()
    w_d = nc.dram_tensor("w_in", [NP, BS], FP32, kind="ExternalInput").ap()
    # weight mats, transposed host-side to [j(part), K, 5, i]
    wa_d = nc.dram_tensor("wa_in", [NP, K, 5, NP], F32R, kind="ExternalInput").ap()
    band_d = nc.dram_tensor("band_in", [NP, 3, NP], FP32, kind="ExternalInput").ap()
    coef_d = nc.dram_tensor("coef_in", [NP, K, NCOEF], FP32,
                            kind="ExternalInput").ap()
    aout_d = nc.dram_tensor("a_out", [NP, BS, N], F32R, kind="ExternalOutput").ap()
    wout_d = nc.dram_tensor("w_out", [NP, BS], FP32, kind="ExternalOutput").ap()

    add = mybir.AluOpType.add
    sub = mybir.AluOpType.subtract
    mult = mybir.AluOpType.mult
    idf = mybir.ActivationFunctionType.Identity

    with tile.TileContext(nc) as tc:
        with (
            tc.tile_pool(name="state", bufs=1) as state,
            tc.tile_pool(name="small", bufs=1) as small,
            tc.tile_pool(name="xascr", bufs=2) as xascr,
            tc.tile_pool(name="duscr", bufs=3) as duscr,
            tc.tile_pool(name="ps", bufs=3, space="PSUM") as psp,
            tc.tile_pool(name="omps", bufs=2, space="PSUM") as ompsp,
        ):
            X = state.tile([NP, BS, N], FP32)
            A = state.tile([NP, BS, N], F32R)
            MU = state.tile([NP, BS, N], F32R)
            WA = state.tile([NP, K, 5, NP], F32R)
            BAND = state.tile([NP, 3, NP], FP32)
            COEF = state.tile([NP, K, NCOEF], FP32)
            Y = small.tile([NP, BS], FP32)
            WOM = small.tile([NP, BS], FP32)
            LAM = small.tile([NP, BS], FP32)
            XA = small.tile([NP, BS], FP32)
            S = small.tile([NP, BS], FP32)
            SEFF = small.tile([NP, BS], FP32)
            OMV = small.tile([NP, BS], FP32)

            # ---- loads ----
            nc.sync.dma_start(out=WA[:], in_=wa_d[:])
            for lo, hi in ((0, 32), (32, 64), (64, 96), (96, 128)):
                nc.sync.dma_start(out=A[lo:hi], in_=a_d[lo:hi])
            for lo, hi in ((0, 32), (32, 64), (64, 96), (96, 128)):
                nc.sync.dma_start(out=X[lo:hi], in_=x_d[lo:hi])
            nc.sync.dma_start(out=BAND[:], in_=band_d[:])
            nc.sync.dma_start(out=COEF[:], in_=coef_d[:])
            nc.sync.dma_start(out=Y[:], in_=y_d[:])
            nc.sync.dma_start(out=WOM[:], in_=w_d[:])
            nc.vector.memset(MU[:].bitcast(FP32), 0.0)
            nc.vector.memset(LAM[:], 0.0)

            blocks = ((0, 50), (64, 114))

            for k in range(K):
                # ---------------- color sweeps ----------------
                for c, (r0, r1) in enumerate(blocks):
                    rows = slice(r0, r1)
                    m1 = 2 * c        # band+diag weights for a
                    m2 = 2 * c + 1    # mu diagonal weights

                    # omega neighbor sum (PE, 0/1 band, tiny free dim)
                    omps = ompsp.tile([NP, BS], FP32, tag="omps")
                    nc.tensor.matmul(omps[:], BAND[:, c, :], WOM[:],
                                     start=True, stop=True)

                    # xa[p, b] = sum_n X*A for the active color rows.
                    # Even sweep on DVE (fused STT+accum); odd sweep product
                    # on GPSIMD with the reduce on ScalarE, to offload DVE.
                    for b in range(BS):
                        scr = xascr.tile([NP, N], FP32, tag="xascr")
                        if c == 0:
                            nc.vector.scalar_tensor_tensor(
                                out=scr[rows], in0=X[rows, b], scalar=1.0,
                                in1=A[rows, b], op0=mult, op1=mult,
                                accum_out=XA[rows, b:b + 1])
                        else:
                            nc.gpsimd.tensor_tensor(
                                scr[rows], X[rows, b], A[rows, b], mult)
                            nc.scalar.activation(
                                out=scr[rows], in_=scr[rows], func=idf,
                                accum_out=XA[rows, b:b + 1])

                    # s = xa + omega - y ; seff = -h1 * s (in 4-b groups so
                    # combines can start before the last xa finishes)
                    for g in range(0, BS, 4):
                        gs = slice(g, g + 4)
                        nc.vector.scalar_tensor_tensor(
                            out=S[rows, gs], in0=XA[rows, gs], scalar=1.0,
                            in1=WOM[rows, gs], op0=mult, op1=add)
                        nc.vector.tensor_tensor(S[rows, gs], S[rows, gs],
                                                Y[rows, gs], sub)
                        nc.vector.tensor_scalar_mul(SEFF[rows, gs],
                                                    S[rows, gs],
                                                    COEF[rows, k, 0:1])

                    # omega update: w' = cw*w + csw*s + clam*lam + cnbw*sum_w
                    nc.vector.tensor_scalar_mul(OMV[rows], WOM[rows],
                                                COEF[rows, k, 1:2])
                    nc.vector.scalar_tensor_tensor(
                        out=OMV[rows], in0=S[rows], scalar=COEF[rows, k, 2:3],
                        in1=OMV[rows], op0=mult, op1=add)
                    if k > 0:
                        nc.vector.scalar_tensor_tensor(
                            out=OMV[rows], in0=LAM[rows],
                            scalar=COEF[rows, k, 3:4],
                            in1=OMV[rows], op0=mult, op1=add)
                    nc.vector.scalar_tensor_tensor(
                        out=WOM[rows], in0=omps[rows],
                        scalar=COEF[rows, k, 4:5],
                        in1=OMV[rows], op0=mult, op1=add)

                    # a-side: psum_b = W1 @ A_b (+ W2 @ MU_b), then
                    # a'_b[rows] = X_b * seff_b + psum_b[rows]
                    for b in range(BS):
                        ps = psp.tile([NP, N], FP32, tag="ps")
                        last = k == 0
                        nc.tensor.matmul(ps[:, 0:NC0], WA[:, k, m1, :],
                                         A[:, b, 0:NC0],
                                         start=True, stop=last)
                        nc.tensor.matmul(ps[:, NC0:N], WA[:, k, m1, :],
                                         A[:, b, NC0:N],
                                         start=True, stop=last)
                        if k > 0:
                            nc.tensor.matmul(ps[:, 0:NC0], WA[:, k, m2, :],
                                             MU[:, b, 0:NC0],
                                             start=False, stop=True)
                            nc.tensor.matmul(ps[:, NC0:N], WA[:, k, m2, :],
                                             MU[:, b, NC0:N],
                                             start=False, stop=True)
                        nc.vector.scalar_tensor_tensor(
                            out=A[rows, b], in0=X[rows, b],
                            scalar=SEFF[rows, b:b + 1],
                            in1=ps[rows], op0=mult, op1=add)

                # ---------------- dual updates ----------------
                if k == K - 1:
                    continue  # final mu/lam are dead state
                # lam' = lam + 4*e4*omega - e4*sum_w
                omps2 = ompsp.tile([NP, BS], FP32, tag="omps")
                nc.tensor.matmul(omps2[:], BAND[:, 2, :], WOM[:],
                                 start=True, stop=True)
                nc.vector.scalar_tensor_tensor(
                    out=LAM[:], in0=WOM[:], scalar=COEF[:, k, 5:6],
                    in1=LAM[:], op0=mult, op1=add)
                nc.vector.scalar_tensor_tensor(
                    out=LAM[:], in0=omps2[:], scalar=COEF[:, k, 6:7],
                    in1=LAM[:], op0=mult, op1=add)
                # mu += W3 @ A: PE matmul; accumulate 4 b's on DVE (straight
                # from PSUM) and 12 via ACT copy + GPSIMD add.
                for b in range(BS):
                    ps = psp.tile([NP, N], FP32, tag="ps")
                    nc.tensor.matmul(ps[:, 0:NC0], WA[:, k, 4, :],
                                     A[:, b, 0:NC0], start=True, stop=True)
                    nc.tensor.matmul(ps[:, NC0:N], WA[:, k, 4, :],
                                     A[:, b, NC0:N], start=True, stop=True)
                    if b % 4 == 0:
                        nc.vector.tensor_tensor(MU[:, b], MU[:, b], ps[:], add)
                    else:
                        scr3 = duscr.tile([NP, N], FP32, tag="duscr")
                        nc.scalar.copy(scr3[:], ps[:])
                        nc.gpsimd.tensor_tensor(MU[:, b], MU[:, b],
                                                scr3[:], add)

            # ---- store results ----
            for b in range(BS):
                nc.sync.dma_start(out=aout_d[:, b], in_=A[:, b])
            nc.sync.dma_start(out=wout_d[:], in_=WOM[:])

    nc.compile()
    _BUILD_CACHE["nc"] = nc
    return nc


def _host_prep(inputs, labels, hyp, a0, omega0, neighbors, colors):
    """Permute nodes into the padded 128-row partition space (color-0 nodes at
    rows 0:50, color-1 at rows 64:114) and fold |h| coefficients into the
    per-iteration node-linear weight matrices."""
    colors = np.asarray(colors)
    neighbors = np.asarray(neighbors)
    perm = np.concatenate([colors[0], colors[1]]).astype(np.int64)
    missing = np.setdiff1d(np.arange(P), perm)
    perm = np.concatenate([perm, missing])          # new idx -> orig node
    ncol = colors.shape[1]
    # new idx -> partition row (color blocks at aligned bases 0 and 64)
    rowmap = np.empty(P, np.int64)
    rowmap[:ncol] = np.arange(ncol)
    rowmap[ncol:2 * ncol] = 64 + np.arange(ncol)
    if 2 * ncol < P:
        rowmap[2 * ncol:] = 114 + np.arange(P - 2 * ncol)
    # orig node -> partition row
    row_of = np.empty(P, np.int64)
    row_of[perm] = rowmap
    nb_row = row_of[neighbors[perm]]                # [P, D] neighbor rows
    habs = np.abs(np.asarray(hyp, np.float64))[:, perm, :]  # [K, P, 6]
    al, be, ga, e3, e4, de = [habs[:, :, j] for j in range(6)]

    blocks = ((0, ncol), (ncol, 2 * ncol))

    WA = np.zeros((K, 5, NP, NP), np.float64)
    for k in range(K):
        for c, (r0, r1) in enumerate(blocks):
            for i in range(r0, r1):
                ri = rowmap[i]
                ab = al[k, i] * be[k, i]
                WA[k, 2 * c, ri, ri] += 1.0 - DEG * ab
                for rj in nb_row[i]:
                    WA[k, 2 * c, rj, ri] += ab
                WA[k, 2 * c + 1, ri, ri] = -DEG * be[k, i]
        for i in range(P):
            ri = rowmap[i]
            WA[k, 4, ri, ri] += DEG * e3[k, i]
            for rj in nb_row[i]:
                WA[k, 4, rj, ri] -= e3[k, i]

    BAND = np.zeros((3, NP, NP), np.float64)
    for c, (r0, r1) in enumerate(blocks):
        for i in range(r0, r1):
            for rj in nb_row[i]:
                BAND[c, rj, rowmap[i]] += 1.0
    for i in range(P):
        for rj in nb_row[i]:
            BAND[2, rj, rowmap[i]] += 1.0

    COEF = np.zeros((NP, K, NCOEF), np.float64)
    COEF[rowmap, :, 0] = -be.T
    COEF[rowmap, :, 1] = (1.0 - DEG * ga * de).T
    COEF[rowmap, :, 2] = -de.T
    COEF[rowmap, :, 3] = (-DEG * de).T
    COEF[rowmap, :, 4] = (ga * de).T
    COEF[rowmap, :, 5] = (DEG * e4).T
    COEF[rowmap, :, 6] = -e4.T

    Xp = np.zeros((NP, B, N), np.float32)
    Xp[rowmap] = np.asarray(inputs)[perm]
    A0p = np.zeros((NP, B, N), np.float32)
    A0p[rowmap] = np.asarray(a0)[perm]
    Yp = np.zeros((NP, B), np.float32)
    Yp[rowmap] = np.asarray(labels)[perm]
    W0p = np.zeros((NP, B), np.float32)
    W0p[rowmap] = np.asarray(omega0)[perm]
    WAh = np.ascontiguousarray(WA.transpose(2, 0, 1, 3), np.float32)
    BANDh = np.ascontiguousarray(BAND.transpose(1, 0, 2), np.float32)
    COEFh = np.ascontiguousarray(COEF, np.float32)
    return perm, rowmap, WAh, BANDh, COEFh, Xp, A0p, Yp, W0p


def kernel(inputs, labels, hyp, a0, omega0, neighbors, colors):
    perm, rowmap, WAh, BANDh, COEFh, Xp, A0p, Yp, W0p = _host_prep(
        inputs, labels, hyp, a0, omega0, neighbors, colors)
    nc = _build_bass()

    in_maps = []
    for core in range(NCORES):
        bsl = slice(core * BS, (core + 1) * BS)
        in_maps.append({
            "x_in": np.ascontiguousarray(Xp[:, bsl, :]),
            "a_in": np.ascontiguousarray(A0p[:, bsl, :]),
            "y_in": np.ascontiguousarray(Yp[:, bsl]),
            "w_in": np.ascontiguousarray(W0p[:, bsl]),
            "wa_in": WAh,
            "band_in": BANDh,
            "coef_in": COEFh,
        })

    res = run_bass_kernel_spmd(nc, in_maps, core_ids=list(range(NCORES)))

    a_full = np.empty((P, B, N), np.float32)
    om_full = np.empty((P, B), np.float32)
    for core, r in enumerate(res.results):
        bsl = slice(core * BS, (core + 1) * BS)
        a_full[perm, bsl, :] = r["a_out"][rowmap]
        om_full[perm, bsl] = r["w_out"][rowmap]
    return a_full, om_full
